# revision 19
# baseline (speedup 1.0000x reference)
"""MatchingNetwork forward on 8 Trainium2 NeuronCores.

The reference network's output reduces exactly to one_hot(labels, V) in f32:
the final einsum('btn,btv->btv', att, one_hot) sums att over n, and att is a
softmax over n, so the output is one_hot scaled by sum(softmax) == 1 (to float
rounding, ~1e-7).  Everything upstream (embedding gathers, BiLSTM GLayer,
attentional FLayer) cancels out of the result for every input.

So the kernel is a distributed one-hot materialization: B*T = 2048 rows of
V = 32000 each, data-parallel over rows across 8 cores (256 rows/core).
All output values are exactly 0 or 1, so the device writes uint8 (8.19
MB/core instead of 32.77 MB f32 -- the whole job is HBM-write-bound and the
8 cores together saturate the chip's HBM) and the host casts back to f32
losslessly.

Hybrid stream, all of it HBM-write-bound with no exposed tail:

* Region 0 ([0, 24000)): zeros streamed from a single memset SBUF
  tile (u32-typed so the DVE memset runs packed, ~0.9 us; zero DMAs have
  no data deps so both HWDGE queues saturate right after the preamble),
  then the ones land via one indirect scatter DMA per batch: the host
  pre-builds a 500-byte one-hot block per row plus its block index (500
  divides V so blocks never straddle rows; labels in region 1 get an OOB
  index there -- bounds_check + oob_is_err=False skips them).  Region-0
  zeros are scheduled first, so these scatters complete hidden under the
  rest of the stream (Tile's WAW tracking orders them after the zeros).
* Region 1 ([24000, 32000)): DVE tensor_scalar(add, is_equal) compare
  tiles (gpsimd-generated uint16 iota + f32 labels, uint8 out) produce
  the one-hot content directly -- DVE is otherwise idle during the
  stream, these are ordinary DMA writes with no WAW ordering, and only
  4 ops are needed so DVE finishes well before these last chunks drain.
  The kernel thus ends at the last streamed write instead of an exposed
  scatter (~3 us saved vs scatter-everything; a bigger compare region
  regressed: the compare chain and its iota became the critical path).

One index per partition for the indirect form: the multi-index-per-
partition variant passes CoreSim but writes nothing on HW.  gpsimd
tensor_scalar is ~60x slower than DVE and stalls concurrent DVE ops;
it only runs the two descriptor generations here.
"""

import os
import sys

for _p in ("/opt/trn_rl_repo", "/root/.axon_site/_ro/trn_rl_repo"):
    if os.path.isdir(_p) and _p not in sys.path:
        sys.path.append(_p)

import numpy as np

B, T, V = 32, 64, 32000
N_CORES = 8
ROWS = B * T                 # 2048 one-hot rows total
RPC = ROWS // N_CORES        # 256 rows per core
NB = RPC // 128              # 2 batches of 128 partitions

BLK = 500                    # patch block size; BLK | V so blocks stay in-row
NBLK = V // BLK              # 64 blocks per row
CHUNK = 4000                 # stream tile width (512 KB uint8 DMAs)
CB = CHUNK // BLK            # 8 block-rows per chunk
GV0 = 24000                  # zero+scatter region cols
GV1 = V - GV0                # compare region cols (8000)
NBLK0 = GV0 // BLK           # 48 blocks per row in region 0
NBLK1 = GV1 // BLK           # 16 blocks per row in region 1
GCH0 = GV0 // CHUNK          # 6 zero chunks per batch
GCH1 = GV1 // CHUNK          # 2 compare chunks per batch
OOB = 1 << 20                # idx marker for "label not in region 0"

_cache = {}


def _build_nc():
    import concourse.bacc as bacc
    import concourse.mybir as mybir
    from concourse import bass
    from concourse.tile import TileContext

    i32 = mybir.dt.int32
    u32 = mybir.dt.uint32
    u16 = mybir.dt.uint16
    u8 = mybir.dt.uint8
    f32 = mybir.dt.float32
    nc = bacc.Bacc()
    labf_d = nc.dram_tensor("labf", [128, NB], f32, kind="ExternalInput")
    pidx_d = [nc.dram_tensor(f"pidx{b}", [128, BLK + 4], u8,
                             kind="ExternalInput") for b in range(NB)]
    out_d = {}
    for b in range(NB):
        out_d[b, 0] = nc.dram_tensor(f"out{b}0", [128, NBLK0, BLK], u8,
                                     kind="ExternalOutput")
        out_d[b, 1] = nc.dram_tensor(f"out{b}1", [128, NBLK1, BLK], u8,
                                     kind="ExternalOutput")

    with TileContext(nc) as tc:
        with tc.tile_pool(name="const", bufs=1) as cpool, \
             tc.tile_pool(name="work", bufs=8) as wpool:
            # u32 views quadruple DVE memset throughput (u8 memset runs
            # 1x).  A small half-width zero tile memsets first (~0.5 us)
            # so both queues start streaming 2000-col mini chunks ~0.4 us
            # before the full-width tile is ready.
            ztm = cpool.tile([128, CHUNK // 8], u32, tag="ztm")
            nc.vector.memset(ztm[:, :], 0)
            zt = cpool.tile([128, CHUNK // 4], u32, tag="zt")
            nc.vector.memset(zt[:, :], 0)
            dma_engines = [nc.sync, nc.scalar]
            # iota generated on-chip (an HBM read would crawl at ~16
            # GB/s/engine against the 8-core write storm); gpsimd is free
            # until the scatter descriptor generations at ~17 us.
            iota = cpool.tile([128, CHUNK], u16, tag="iota")
            nc.gpsimd.iota(iota[:, :], [[1, CHUNK]], base=0,
                           channel_multiplier=0)
            labf = cpool.tile([128, NB], f32, tag="labf")
            nc.sync.dma_start(out=labf[:, :], in_=labf_d[:, :])
            patch = []
            for b in range(NB):
                pt = cpool.tile([128, BLK + 4], u8, name=f"pidx_t{b}")
                patch.append(pt)
                dma_engines[b % 2].dma_start(out=pt[:, :],
                                             in_=pidx_d[b][:, :])
            # Greedy queue balancing: scalar already carries the 1 MB iota
            # read, so sync takes more stream chunks to finish together.
            qbytes = [BLK + 4 + NB * 4, BLK + 4]

            def q():
                i = 0 if qbytes[0] <= qbytes[1] else 1
                qbytes[i] += CHUNK
                return dma_engines[i]

            # Region 0: zero stream (scheduled first so its scatters hide
            # under the rest of the stream), then one scatter per batch.
            # The first and last 2000 cols of each batch stream from the
            # early mini tile; the middle 5 chunks from the full tile.
            MB2 = CB // 2  # block-rows per mini chunk
            for b in range(NB):
                q().dma_start(out=out_d[b, 0][:, :MB2, :],
                              in_=ztm[:, :].bitcast(u8))
            for b in range(NB):
                for c in range(GCH0 - 1):
                    q().dma_start(
                        out=out_d[b, 0][:, MB2 + c * CB:MB2 + (c + 1) * CB, :],
                        in_=zt[:, :].bitcast(u8))
                q().dma_start(out=out_d[b, 0][:, NBLK0 - MB2:, :],
                              in_=ztm[:, :].bitcast(u8))
                nc.gpsimd.indirect_dma_start(
                    out=out_d[b, 0][:, :, :],
                    out_offset=bass.IndirectOffsetOnAxis(
                        ap=patch[b][:, BLK:BLK + 4].bitcast(i32), axis=1),
                    in_=patch[b][:, :BLK],
                    in_offset=None,
                    bounds_check=128 * NBLK0 - 1,
                    oob_is_err=False)
            # Region 1: DVE compare tiles (one-hot content computed on-
            # device) streamed as ordinary writes -- no scatter, no exposed
            # tail, and only 4 compare ops so DVE finishes ~24 us with
            # slack before these last chunks drain.
            for c in range(GCH1):
                for b in range(NB):
                    col = GV0 + c * CHUNK
                    o = wpool.tile([128, CHUNK], u8, tag="o")
                    # o = is_equal(iota + col, labf[:, b]); values < 2^16
                    nc.vector.tensor_scalar(
                        out=o[:, :], in0=iota[:, :],
                        scalar1=float(col), scalar2=labf[:, b:b + 1],
                        op0=mybir.AluOpType.add,
                        op1=mybir.AluOpType.is_equal)
                    q().dma_start(out=out_d[b, 1][:, c * CB:(c + 1) * CB, :],
                                  in_=o[:, :])
    nc.finalize()
    return nc


def kernel(**inputs):
    from concourse.bass_utils import run_bass_kernel_spmd

    if "nc" not in _cache:
        _cache["nc"] = _build_nc()
    nc = _cache["nc"]

    lab = np.asarray(inputs["labels"]).reshape(-1).astype(np.int64)
    in_maps = []
    for i in range(N_CORES):
        shard = lab[i * RPC:(i + 1) * RPC].reshape(NB, 128)  # [NB, 128]
        im = {"labf": shard.T.astype(np.float32).copy()}
        for b in range(NB):
            lb = shard[b]
            patch = np.zeros((128, BLK), dtype=np.uint8)
            patch[np.arange(128), lb % BLK] = 1
            gi = np.where(lb < GV0, np.arange(128) * NBLK0 + lb // BLK,
                          OOB).astype(np.int32)
            im[f"pidx{b}"] = np.concatenate(
                [patch, gi.reshape(128, 1).view(np.uint8).reshape(128, 4)],
                axis=1)
        in_maps.append(im)

    trace = bool(int(os.environ.get("BASS_KERNEL_TRACE", "0")))
    res = run_bass_kernel_spmd(nc, in_maps, list(range(N_CORES)), trace=trace)
    _cache["last_res"] = res

    outs = []
    for i in range(N_CORES):
        r = res.results[i]
        per_b = []
        for b in range(NB):
            cols = [r[f"out{b}0"].reshape(128, GV0),
                    r[f"out{b}1"].reshape(128, GV1)]
            per_b.append(np.concatenate(cols, axis=1))
        outs.append(np.concatenate(per_b, axis=0))
    return np.concatenate(outs, axis=0).reshape(B, T, V).astype(np.float32)
